# revision 7
# baseline (speedup 1.0000x reference)
"""Cross-attention (GQA + RoPE + qk-RMSNorm + masked softmax) Trainium2 kernel.

Sharding: 8 cores = B(2) x KV-groups(4). Each core computes its batch's
4 query heads (one KV group) end-to-end, producing a partial output
(row-parallel wo); host sums the 4 partials per batch.

All matmuls run as float32r (TF32-like: fp32 with 11-bit mantissa, RTN,
fp32 accumulate in PSUM) at 1 cycle/row.
"""
import sys

sys.path.insert(0, "/opt/trn_rl_repo")

import numpy as np

import concourse.bass as bass  # noqa: F401
import concourse.mybir as mybir
import concourse.tile as tile
from concourse import bacc
from concourse.bass_utils import run_bass_kernel_spmd
from concourse.masks import make_identity

DIM, H, KVH, HD = 2048, 16, 4, 128
B, T, S = 2, 2048, 2048
G = H // KVH          # heads per kv group = 4
GD = G * HD           # 512, per-core head dims
EPS = 1.1920929e-07
SCALE = 1.0 / np.sqrt(HD)
P = 128
NT = T // P           # 16 q chunks
NS = S // P           # 16 s chunks
ND = DIM // P         # 16 d chunks
MASK_NEG = -30.0

F32 = mybir.dt.float32
F32R = mybir.dt.float32r

_cache = {}


def _build(active):
    """Build + compile the SPMD program for a given tuple of active s-chunks."""
    nsa = len(active)
    nc = bacc.Bacc(None, target_bir_lowering=False, debug=False)
    names = {}

    with tile.TileContext(nc) as tc:
        with tc.tile_pool(name="dram", bufs=1, space="DRAM") as dram:
            d_xt = dram.tile([NT, P, ND, P], F32R, kind="ExternalInput", name="xt")
            d_ctxt = dram.tile([nsa, P, ND, P], F32R, kind="ExternalInput", name="ctxt")
            d_wqt = dram.tile([P, ND, GD], F32R, kind="ExternalInput", name="wqt")
            d_wkvt = dram.tile([P, ND, 2 * P], F32R, kind="ExternalInput", name="wkvt")
            d_wot = dram.tile([P, G, DIM], F32R, kind="ExternalInput", name="wot")
            d_cos4 = dram.tile([NT, P, GD], F32, kind="ExternalInput", name="cos4")
            d_sin4 = dram.tile([NT, P, GD], F32, kind="ExternalInput", name="sin4")
            d_maskb = dram.tile([P, NS], F32, kind="ExternalInput", name="maskb")
            d_g128 = dram.tile([P, P], F32, kind="ExternalInput", name="g128")
            d_out = dram.tile([T, DIM], F32, kind="ExternalOutput", name="out")
            for k, t in [("xt", d_xt), ("ctxt", d_ctxt), ("wqt", d_wqt),
                         ("wkvt", d_wkvt), ("wot", d_wot), ("cos4", d_cos4),
                         ("sin4", d_sin4), ("maskb", d_maskb), ("g128", d_g128),
                         ("out", d_out)]:
                names[k] = t.tensor.name

            cpool = tc.tile_pool(name="const", bufs=1)
            const = cpool.__enter__()
            ident = const.tile([P, P], F32)
            make_identity(nc, ident)
            ones_f = const.tile([P, P], F32)
            nc.any.memset(ones_f[:], 1.0)
            ones_s = const.tile([P, 1], F32R)
            nc.vector.tensor_copy(ones_s[:], ones_f[:, 0:1])
            ones_k1 = const.tile([1, P], F32R)
            nc.vector.tensor_copy(ones_k1[:], ones_f[0:1, :])
            g128 = const.tile([P, P], F32)
            nc.sync.dma_start(out=g128[:], in_=d_g128[:])
            maskb = const.tile([P, NS], F32)
            nc.sync.dma_start(out=maskb[:], in_=d_maskb[:])
            eps_t = const.tile([P, 1], F32)
            nc.any.memset(eps_t[:], float(EPS))
            # persistent K^T and V for the active s-chunks
            kt_big = const.tile([P, nsa * P], F32R)
            v_big = const.tile([P, nsa * P], F32R)
            # resident weights
            wqt = const.tile([P, ND, GD], F32R)
            nc.sync.dma_start(out=wqt[:], in_=d_wqt[:])
            wkvt = const.tile([P, ND, 2 * P], F32R)
            nc.sync.dma_start(out=wkvt[:], in_=d_wkvt[:])
            wot = const.tile([P, G, DIM], F32R)
            nc.sync.dma_start(out=wot[:], in_=d_wot[:])

            # ---------------- Phase 0: K/V projection + k-norm + K^T ----------
            with tc.tile_pool(name="p0_sb", bufs=3) as p0sb, \
                 tc.tile_pool(name="p0_work", bufs=2) as p0w, \
                 tc.tile_pool(name="p0_ps", bufs=2, space="PSUM") as p0ps, \
                 tc.tile_pool(name="p0_tps", bufs=2, space="PSUM") as p0tps:
                for si in range(nsa):
                    ctx_t = p0sb.tile([P, ND, P], F32R, tag="ctx")
                    nc.sync.dma_start(out=ctx_t[:], in_=d_ctxt[si])
                    kv_ps = p0ps.tile([P, 2 * P], F32, tag="kv")
                    for dc in range(ND):
                        nc.tensor.matmul(kv_ps[:], ctx_t[:, dc, :], wkvt[:, dc, :],
                                         start=(dc == 0), stop=(dc == ND - 1))
                    # k RMSNorm (free dim = hd), fold norm weights g128
                    ksq = p0w.tile([P, P], F32, tag="ksq")
                    nc.scalar.square(ksq[:], kv_ps[:, 0:P])
                    ssq = p0w.tile([P, 1], F32, tag="ssq")
                    nc.vector.reduce_sum(ssq[:], ksq[:], axis=mybir.AxisListType.X)
                    rms = p0w.tile([P, 1], F32, tag="rms")
                    nc.scalar.activation(rms[:], ssq[:],
                                         mybir.ActivationFunctionType.Sqrt,
                                         bias=eps_t[:], scale=1.0 / HD)
                    rinv = p0w.tile([P, 1], F32, tag="rinv")
                    nc.vector.reciprocal(rinv[:], rms[:])
                    kn = p0w.tile([P, P], F32, tag="kn")
                    nc.vector.tensor_scalar_mul(kn[:], kv_ps[:, 0:P], rinv[:])
                    kn2 = p0w.tile([P, P], F32, tag="kn2")
                    nc.vector.tensor_mul(kn2[:], kn[:], g128[:])
                    # V straight out
                    nc.any.tensor_copy(v_big[:, si * P:(si + 1) * P],
                                       kv_ps[:, P:2 * P])
                    # K^T via PE transpose
                    kt_ps = p0tps.tile([P, P], F32, tag="ktp")
                    nc.tensor.transpose(kt_ps[:], kn2[:], ident[:])
                    nc.any.tensor_copy(kt_big[:, si * P:(si + 1) * P], kt_ps[:])

            # ---------------- Fused Q/attn/wo loop over q chunks --------------
            with tc.tile_pool(name="str_sb", bufs=3) as strm, \
                 tc.tile_pool(name="wk_sb", bufs=2) as wk, \
                 tc.tile_pool(name="p_sb", bufs=3) as pp, \
                 tc.tile_pool(name="o_sb", bufs=2) as osb, \
                 tc.tile_pool(name="ps_a", bufs=3, space="PSUM") as psa, \
                 tc.tile_pool(name="ps_b", bufs=2, space="PSUM") as psb, \
                 tc.tile_pool(name="ps_att", bufs=2, space="PSUM") as psatt, \
                 tc.tile_pool(name="ps_den", bufs=1, space="PSUM") as psden:
                for tc_i in range(NT):
                    x_t = strm.tile([P, ND, P], F32R, tag="xs")
                    nc.sync.dma_start(out=x_t[:], in_=d_xt[tc_i])
                    cos_t = strm.tile([P, GD], F32, tag="cos")
                    nc.sync.dma_start(out=cos_t[:], in_=d_cos4[tc_i])
                    sin_t = strm.tile([P, GD], F32, tag="sin")
                    nc.sync.dma_start(out=sin_t[:], in_=d_sin4[tc_i])

                    q_ps = psa.tile([P, GD], F32, tag="mm512")
                    for dc in range(ND):
                        nc.tensor.matmul(q_ps[:], x_t[:, dc, :], wqt[:, dc, :],
                                         start=(dc == 0), stop=(dc == ND - 1))
                    # RoPE: q2 = q*cos + rotate_half(q)*sin
                    qc = wk.tile([P, GD], F32, tag="qc")
                    nc.vector.tensor_mul(qc[:], q_ps[:], cos_t[:])
                    rot = wk.tile([P, GD], F32, tag="rot")
                    rot3 = rot[:].rearrange("p (a two) -> p a two", two=2)
                    q3 = q_ps[:].rearrange("p (a two) -> p a two", two=2)
                    nc.vector.tensor_scalar_mul(rot3[:, :, 0], q3[:, :, 1], -1.0)
                    nc.vector.tensor_copy(rot3[:, :, 1], q3[:, :, 0])
                    nc.vector.tensor_mul(rot[:], rot[:], sin_t[:])
                    q2 = wk.tile([P, GD], F32, tag="q2")
                    nc.vector.tensor_add(q2[:], qc[:], rot[:])
                    # q RMSNorm per head
                    qsq = wk.tile([P, GD], F32, tag="qsq")
                    nc.vector.tensor_mul(qsq[:], q2[:], q2[:])
                    ssq4 = wk.tile([P, G], F32, tag="ssq4")
                    nc.vector.reduce_sum(ssq4[:],
                                         qsq[:].rearrange("p (h r) -> p h r", r=P),
                                         axis=mybir.AxisListType.X)
                    rms4 = wk.tile([P, G], F32, tag="rms4")
                    nc.scalar.activation(rms4[:], ssq4[:],
                                         mybir.ActivationFunctionType.Sqrt,
                                         bias=eps_t[:], scale=1.0 / HD)
                    rinv4 = wk.tile([P, G], F32, tag="rinv4")
                    nc.vector.reciprocal(rinv4[:], rms4[:])
                    qn = wk.tile([P, GD], F32, tag="qn")
                    for h in range(G):
                        nc.vector.tensor_scalar_mul(qn[:, h * P:(h + 1) * P],
                                                    q2[:, h * P:(h + 1) * P],
                                                    rinv4[:, h:h + 1])
                    # Q^T (per head) -> [hd, (h, t)]
                    qt_ps = psb.tile([P, GD], F32, tag="sc512")
                    for h in range(G):
                        nc.tensor.transpose(qt_ps[:, h * P:(h + 1) * P],
                                            qn[:, h * P:(h + 1) * P], ident[:])
                    qt_sb = wk.tile([P, GD], F32R, tag="qt")
                    nc.any.tensor_copy(qt_sb[:], qt_ps[:])

                    # attention: scoresT [s, (h,t)] per s-chunk; exp; PV; denom
                    att_ps = psatt.tile([P, GD], F32, tag="att")
                    den_ps = psden.tile([1, GD], F32, tag="den")
                    for si in range(nsa):
                        sc_ps = psb.tile([P, GD], F32, tag="sc512")
                        nc.tensor.matmul(sc_ps[:], kt_big[:, si * P:(si + 1) * P],
                                         qt_sb[:], start=True, stop=True)
                        p_t = pp.tile([P, GD], F32R, tag="p")
                        nc.scalar.activation(p_t[:], sc_ps[:],
                                             mybir.ActivationFunctionType.Exp,
                                             bias=maskb[:, active[si]:active[si] + 1],
                                             scale=float(SCALE))
                        nc.tensor.matmul(att_ps[:], v_big[:, si * P:(si + 1) * P],
                                         p_t[:], start=(si == 0), stop=(si == nsa - 1))
                        nc.tensor.matmul(den_ps[:], ones_s[:], p_t[:],
                                         start=(si == 0), stop=(si == nsa - 1))
                    recip = wk.tile([1, GD], F32R, tag="recip")
                    with nc.allow_low_precision(reason="fp32r feed to replicate mm"):
                        nc.vector.reciprocal(recip[:], den_ps[:])
                    rep_ps = psa.tile([P, GD], F32, tag="mm512")
                    nc.tensor.matmul(rep_ps[:], ones_k1[:], recip[:],
                                     start=True, stop=True)
                    rep_sb = wk.tile([P, GD], F32, tag="rep")
                    nc.any.tensor_copy(rep_sb[:], rep_ps[:])
                    attn = wk.tile([P, GD], F32R, tag="attn")
                    nc.vector.tensor_mul(attn[:], att_ps[:], rep_sb[:])

                    # wo: out[t, :] += attn_h @ woT
                    out_sb = osb.tile([P, DIM], F32, tag="out")
                    for nb in range(4):
                        wo_ps = psa.tile([P, 512], F32, tag="mm512")
                        for h in range(G):
                            nc.tensor.matmul(wo_ps[:],
                                             attn[:, h * P:(h + 1) * P],
                                             wot[:, h, nb * 512:(nb + 1) * 512],
                                             start=(h == 0), stop=(h == G - 1))
                        nc.any.tensor_copy(out_sb[:, nb * 512:(nb + 1) * 512],
                                           wo_ps[:])
                    nc.sync.dma_start(out=d_out[tc_i * P:(tc_i + 1) * P, :],
                                      in_=out_sb[:])
            cpool.__exit__(None, None, None)

    nc.compile()
    return nc, names


def _prep_core(b, g, x, context, cos, sin, maskb_col, wq, wkv, wo, gvec, active):
    nsa = len(active)
    # xt[tc, i, dc, j] = x[b][tc*P+j, dc*P+i]
    xt = np.ascontiguousarray(
        x[b].reshape(NT, P, ND, P).transpose(0, 3, 2, 1))
    ctx4 = context[b].reshape(NS, P, ND, P)  # [sc, j, dc, i]
    ctxt = np.ascontiguousarray(ctx4[list(active)].transpose(0, 3, 2, 1))
    wq_g = wq[g * GD:(g + 1) * GD, :]          # [512, 2048]
    wqt = np.ascontiguousarray(
        wq_g.reshape(GD, ND, P).transpose(2, 1, 0))  # [i, dc, hd]
    wk_g = wkv[g * P:(g + 1) * P, :]           # [128, 2048]
    wv_g = wkv[KVH * HD + g * P: KVH * HD + (g + 1) * P, :]
    wkv_g = np.concatenate([wk_g, wv_g], 0)    # [256, 2048]
    wkvt = np.ascontiguousarray(
        wkv_g.reshape(2 * P, ND, P).transpose(2, 1, 0))  # [i, dc, 256]
    wo_g = wo[:, g * GD:(g + 1) * GD]          # [2048(n), 512]
    wot = np.ascontiguousarray(
        wo_g.reshape(DIM, G, P).transpose(2, 1, 0))  # [r, h, n]
    cos4 = np.ascontiguousarray(
        np.tile(cos.reshape(NT, P, 1, HD), (1, 1, G, 1)).reshape(NT, P, GD))
    sin4 = np.ascontiguousarray(
        np.tile(sin.reshape(NT, P, 1, HD), (1, 1, G, 1)).reshape(NT, P, GD))
    g128 = np.ascontiguousarray(np.tile(gvec[None, :], (P, 1)))
    return {
        "xt": xt.astype(np.float32), "ctxt": ctxt.astype(np.float32),
        "wqt": wqt.astype(np.float32), "wkvt": wkvt.astype(np.float32),
        "wot": wot.astype(np.float32), "cos4": cos4.astype(np.float32),
        "sin4": sin4.astype(np.float32), "maskb": maskb_col.astype(np.float32),
        "g128": g128.astype(np.float32),
    }


def kernel(x, context, freqs_cos, freqs_sin, context_mask, wq, wkv, wo,
           q_norm_w, k_norm_w, _trace=False):
    x = np.asarray(x, np.float32)
    context = np.asarray(context, np.float32)
    cos = np.asarray(freqs_cos, np.float32).reshape(T, HD)
    sin = np.asarray(freqs_sin, np.float32).reshape(T, HD)
    mask = np.asarray(context_mask).astype(bool)
    wq = np.asarray(wq, np.float32)
    wkv = np.asarray(wkv, np.float32)
    wo = np.asarray(wo, np.float32)
    gvec = (np.asarray(q_norm_w, np.float32) * np.asarray(k_norm_w, np.float32))

    mask_any = mask.any(axis=0)  # union over batches
    active = tuple(si for si in range(NS) if mask_any[si * P:(si + 1) * P].any())
    if not active:
        active = (0,)

    key = active
    if key not in _cache:
        _cache[key] = _build(active)
    nc, names = _cache[key]

    in_maps = []
    for core in range(8):
        b, g = core // KVH, core % KVH
        maskb_col = np.where(mask[b], 0.0, MASK_NEG).astype(np.float32) \
            .reshape(NS, P).T  # [P, NS] column si = chunk si
        m = _prep_core(b, g, x, context, cos, sin,
                       np.ascontiguousarray(maskb_col), wq, wkv, wo, gvec, active)
        in_maps.append({names[k]: v for k, v in m.items()})

    res = run_bass_kernel_spmd(nc, in_maps, core_ids=list(range(8)), trace=_trace)
    out = np.zeros((B, T, DIM), np.float32)
    for core in range(8):
        out[core // KVH] += res.results[core][names["out"]]
    if _trace:
        kernel._last_result = res
    return out


# revision 8
# speedup vs baseline: 106.1033x; 106.1033x over previous
"""Cross-attention (GQA + RoPE + qk-RMSNorm + masked softmax) Trainium2 kernel.

Sharding: 8 cores = B(2) x KV-groups(4). Each core computes its batch's
4 query heads (one KV group) end-to-end, producing a partial output
(row-parallel wo); host sums the 4 partials per batch.

All matmuls run as float32r (TF32-like: fp32 with 11-bit mantissa, RTN,
fp32 accumulate in PSUM) at 1 cycle/row.
"""
import sys

sys.path.insert(0, "/opt/trn_rl_repo")

import numpy as np

import concourse.bass as bass  # noqa: F401
import concourse.mybir as mybir
import concourse.tile as tile
from concourse import bacc
from concourse.bass_utils import run_bass_kernel_spmd
from concourse.masks import make_identity

DIM, H, KVH, HD = 2048, 16, 4, 128
B, T, S = 2, 2048, 2048
G = H // KVH          # heads per kv group = 4
GD = G * HD           # 512, per-core head dims
EPS = 1.1920929e-07
SCALE = 1.0 / np.sqrt(HD)
P = 128
NT = T // P           # 16 q chunks
NS = S // P           # 16 s chunks
ND = DIM // P         # 16 d chunks
MASK_NEG = -30.0

F32 = mybir.dt.float32
F32R = mybir.dt.float32r

_cache = {}


def _build(active, repeat=1):
    """Build + compile the SPMD program for a given tuple of active s-chunks.

    repeat>1 wraps the whole compute body in an on-device loop (for timing)."""
    nsa = len(active)
    nc = bacc.Bacc(None, target_bir_lowering=False, debug=False)
    names = {}

    with tile.TileContext(nc) as tc:
        with tc.tile_pool(name="dram", bufs=1, space="DRAM") as dram:
            d_xt = dram.tile([NT, P, ND, P], F32R, kind="ExternalInput", name="xt")
            d_ctxt = dram.tile([nsa, P, ND, P], F32R, kind="ExternalInput", name="ctxt")
            d_wqt = dram.tile([P, ND, GD], F32R, kind="ExternalInput", name="wqt")
            d_wkvt = dram.tile([P, ND, 2 * P], F32R, kind="ExternalInput", name="wkvt")
            d_wot = dram.tile([P, G, DIM], F32R, kind="ExternalInput", name="wot")
            d_cos4 = dram.tile([NT, P, GD], F32, kind="ExternalInput", name="cos4")
            d_sin4 = dram.tile([NT, P, GD], F32, kind="ExternalInput", name="sin4")
            d_maskb = dram.tile([P, NS], F32, kind="ExternalInput", name="maskb")
            d_g128 = dram.tile([P, P], F32, kind="ExternalInput", name="g128")
            d_out = dram.tile([T, DIM], F32, kind="ExternalOutput", name="out")
            for k, t in [("xt", d_xt), ("ctxt", d_ctxt), ("wqt", d_wqt),
                         ("wkvt", d_wkvt), ("wot", d_wot), ("cos4", d_cos4),
                         ("sin4", d_sin4), ("maskb", d_maskb), ("g128", d_g128),
                         ("out", d_out)]:
                names[k] = t.tensor.name

            cpool = tc.tile_pool(name="const", bufs=1)
            const = cpool.__enter__()
            ident = const.tile([P, P], F32)
            make_identity(nc, ident)
            ones_f = const.tile([P, P], F32)
            nc.any.memset(ones_f[:], 1.0)
            ones_s = const.tile([P, 1], F32R)
            nc.vector.tensor_copy(ones_s[:], ones_f[:, 0:1])
            ones_k1 = const.tile([1, P], F32R)
            nc.vector.tensor_copy(ones_k1[:], ones_f[0:1, :])
            g128 = const.tile([P, P], F32)
            nc.sync.dma_start(out=g128[:], in_=d_g128[:])
            maskb = const.tile([P, NS], F32)
            nc.sync.dma_start(out=maskb[:], in_=d_maskb[:])
            eps_t = const.tile([P, 1], F32)
            nc.any.memset(eps_t[:], float(EPS))
            # persistent K^T and V for the active s-chunks
            kt_big = const.tile([P, nsa * P], F32R)
            v_big = const.tile([P, nsa * P], F32R)
            # resident weights
            wqt = const.tile([P, ND, GD], F32R)
            nc.sync.dma_start(out=wqt[:], in_=d_wqt[:])
            wkvt = const.tile([P, ND, 2 * P], F32R)
            nc.sync.dma_start(out=wkvt[:], in_=d_wkvt[:])
            wot = const.tile([P, G, DIM], F32R)
            nc.sync.dma_start(out=wot[:], in_=d_wot[:])

            rep_ctx = tc.For_i(0, repeat, 1) if repeat > 1 else None
            if rep_ctx is not None:
                rep_ctx.__enter__()

            # ---------------- Phase 0: K/V projection + k-norm + K^T ----------
            with tc.tile_pool(name="p0_sb", bufs=3) as p0sb, \
                 tc.tile_pool(name="p0_work", bufs=2) as p0w, \
                 tc.tile_pool(name="p0_ps", bufs=2, space="PSUM") as p0ps, \
                 tc.tile_pool(name="p0_tps", bufs=2, space="PSUM") as p0tps:
                for si in range(nsa):
                    ctx_t = p0sb.tile([P, ND, P], F32R, tag="ctx")
                    nc.sync.dma_start(out=ctx_t[:], in_=d_ctxt[si])
                    kv_ps = p0ps.tile([P, 2 * P], F32, tag="kv")
                    for dc in range(ND):
                        nc.tensor.matmul(kv_ps[:], ctx_t[:, dc, :], wkvt[:, dc, :],
                                         start=(dc == 0), stop=(dc == ND - 1))
                    # k RMSNorm (free dim = hd), fold norm weights g128
                    ksq = p0w.tile([P, P], F32, tag="ksq")
                    nc.scalar.square(ksq[:], kv_ps[:, 0:P])
                    ssq = p0w.tile([P, 1], F32, tag="ssq")
                    nc.vector.reduce_sum(ssq[:], ksq[:], axis=mybir.AxisListType.X)
                    rms = p0w.tile([P, 1], F32, tag="rms")
                    nc.scalar.activation(rms[:], ssq[:],
                                         mybir.ActivationFunctionType.Sqrt,
                                         bias=eps_t[:], scale=1.0 / HD)
                    rinv = p0w.tile([P, 1], F32, tag="rinv")
                    nc.vector.reciprocal(rinv[:], rms[:])
                    kn = p0w.tile([P, P], F32, tag="kn")
                    nc.vector.tensor_scalar_mul(kn[:], kv_ps[:, 0:P], rinv[:])
                    kn2 = p0w.tile([P, P], F32, tag="kn2")
                    nc.vector.tensor_mul(kn2[:], kn[:], g128[:])
                    # V straight out
                    nc.any.tensor_copy(v_big[:, si * P:(si + 1) * P],
                                       kv_ps[:, P:2 * P])
                    # K^T via PE transpose
                    kt_ps = p0tps.tile([P, P], F32, tag="ktp")
                    nc.tensor.transpose(kt_ps[:], kn2[:], ident[:])
                    nc.any.tensor_copy(kt_big[:, si * P:(si + 1) * P], kt_ps[:])

            # ---------------- Fused Q/attn/wo loop over q chunks --------------
            with tc.tile_pool(name="str_sb", bufs=3) as strm, \
                 tc.tile_pool(name="wk_sb", bufs=2) as wk, \
                 tc.tile_pool(name="p_sb", bufs=3) as pp, \
                 tc.tile_pool(name="o_sb", bufs=2) as osb, \
                 tc.tile_pool(name="ps_a", bufs=3, space="PSUM") as psa, \
                 tc.tile_pool(name="ps_b", bufs=2, space="PSUM") as psb, \
                 tc.tile_pool(name="ps_att", bufs=2, space="PSUM") as psatt, \
                 tc.tile_pool(name="ps_den", bufs=1, space="PSUM") as psden:
                for tc_i in range(NT):
                    x_t = strm.tile([P, ND, P], F32R, tag="xs")
                    nc.sync.dma_start(out=x_t[:], in_=d_xt[tc_i])
                    cos_t = strm.tile([P, GD], F32, tag="cos")
                    nc.sync.dma_start(out=cos_t[:], in_=d_cos4[tc_i])
                    sin_t = strm.tile([P, GD], F32, tag="sin")
                    nc.sync.dma_start(out=sin_t[:], in_=d_sin4[tc_i])

                    q_ps = psa.tile([P, GD], F32, tag="mm512")
                    for dc in range(ND):
                        nc.tensor.matmul(q_ps[:], x_t[:, dc, :], wqt[:, dc, :],
                                         start=(dc == 0), stop=(dc == ND - 1))
                    # RoPE: q2 = q*cos + rotate_half(q)*sin
                    qc = wk.tile([P, GD], F32, tag="qc")
                    nc.vector.tensor_mul(qc[:], q_ps[:], cos_t[:])
                    rot = wk.tile([P, GD], F32, tag="rot")
                    rot3 = rot[:].rearrange("p (a two) -> p a two", two=2)
                    q3 = q_ps[:].rearrange("p (a two) -> p a two", two=2)
                    nc.vector.tensor_scalar_mul(rot3[:, :, 0], q3[:, :, 1], -1.0)
                    nc.vector.tensor_copy(rot3[:, :, 1], q3[:, :, 0])
                    nc.vector.tensor_mul(rot[:], rot[:], sin_t[:])
                    q2 = wk.tile([P, GD], F32, tag="q2")
                    nc.vector.tensor_add(q2[:], qc[:], rot[:])
                    # q RMSNorm per head
                    qsq = wk.tile([P, GD], F32, tag="qsq")
                    nc.vector.tensor_mul(qsq[:], q2[:], q2[:])
                    ssq4 = wk.tile([P, G], F32, tag="ssq4")
                    nc.vector.reduce_sum(ssq4[:],
                                         qsq[:].rearrange("p (h r) -> p h r", r=P),
                                         axis=mybir.AxisListType.X)
                    rms4 = wk.tile([P, G], F32, tag="rms4")
                    nc.scalar.activation(rms4[:], ssq4[:],
                                         mybir.ActivationFunctionType.Sqrt,
                                         bias=eps_t[:], scale=1.0 / HD)
                    rinv4 = wk.tile([P, G], F32, tag="rinv4")
                    nc.vector.reciprocal(rinv4[:], rms4[:])
                    qn = wk.tile([P, GD], F32, tag="qn")
                    for h in range(G):
                        nc.vector.tensor_scalar_mul(qn[:, h * P:(h + 1) * P],
                                                    q2[:, h * P:(h + 1) * P],
                                                    rinv4[:, h:h + 1])
                    # Q^T (per head) -> [hd, (h, t)]
                    qt_ps = psb.tile([P, GD], F32, tag="sc512")
                    for h in range(G):
                        nc.tensor.transpose(qt_ps[:, h * P:(h + 1) * P],
                                            qn[:, h * P:(h + 1) * P], ident[:])
                    qt_sb = wk.tile([P, GD], F32R, tag="qt")
                    nc.any.tensor_copy(qt_sb[:], qt_ps[:])

                    # attention: scoresT [s, (h,t)] per s-chunk; exp; PV; denom
                    att_ps = psatt.tile([P, GD], F32, tag="att")
                    den_ps = psden.tile([1, GD], F32, tag="den")
                    for si in range(nsa):
                        sc_ps = psb.tile([P, GD], F32, tag="sc512")
                        nc.tensor.matmul(sc_ps[:], kt_big[:, si * P:(si + 1) * P],
                                         qt_sb[:], start=True, stop=True)
                        p_t = pp.tile([P, GD], F32R, tag="p")
                        nc.scalar.activation(p_t[:], sc_ps[:],
                                             mybir.ActivationFunctionType.Exp,
                                             bias=maskb[:, active[si]:active[si] + 1],
                                             scale=float(SCALE))
                        nc.tensor.matmul(att_ps[:], v_big[:, si * P:(si + 1) * P],
                                         p_t[:], start=(si == 0), stop=(si == nsa - 1))
                        nc.tensor.matmul(den_ps[:], ones_s[:], p_t[:],
                                         start=(si == 0), stop=(si == nsa - 1))
                    recip = wk.tile([1, GD], F32R, tag="recip")
                    with nc.allow_low_precision(reason="fp32r feed to replicate mm"):
                        nc.vector.reciprocal(recip[:], den_ps[:])
                    rep_ps = psa.tile([P, GD], F32, tag="mm512")
                    nc.tensor.matmul(rep_ps[:], ones_k1[:], recip[:],
                                     start=True, stop=True)
                    rep_sb = wk.tile([P, GD], F32, tag="rep")
                    nc.any.tensor_copy(rep_sb[:], rep_ps[:])
                    attn = wk.tile([P, GD], F32R, tag="attn")
                    nc.vector.tensor_mul(attn[:], att_ps[:], rep_sb[:])

                    # wo: out[t, :] += attn_h @ woT
                    out_sb = osb.tile([P, DIM], F32, tag="out")
                    for nb in range(4):
                        wo_ps = psa.tile([P, 512], F32, tag="mm512")
                        for h in range(G):
                            nc.tensor.matmul(wo_ps[:],
                                             attn[:, h * P:(h + 1) * P],
                                             wot[:, h, nb * 512:(nb + 1) * 512],
                                             start=(h == 0), stop=(h == G - 1))
                        nc.any.tensor_copy(out_sb[:, nb * 512:(nb + 1) * 512],
                                           wo_ps[:])
                    nc.sync.dma_start(out=d_out[tc_i * P:(tc_i + 1) * P, :],
                                      in_=out_sb[:])
            if rep_ctx is not None:
                rep_ctx.__exit__(None, None, None)
            cpool.__exit__(None, None, None)

    nc.compile()
    return nc, names


def _prep_core(b, g, x, context, cos, sin, maskb_col, wq, wkv, wo, gvec, active):
    nsa = len(active)
    # xt[tc, i, dc, j] = x[b][tc*P+j, dc*P+i]
    xt = np.ascontiguousarray(
        x[b].reshape(NT, P, ND, P).transpose(0, 3, 2, 1))
    ctx4 = context[b].reshape(NS, P, ND, P)  # [sc, j, dc, i]
    ctxt = np.ascontiguousarray(ctx4[list(active)].transpose(0, 3, 2, 1))
    wq_g = wq[g * GD:(g + 1) * GD, :]          # [512, 2048]
    wqt = np.ascontiguousarray(
        wq_g.reshape(GD, ND, P).transpose(2, 1, 0))  # [i, dc, hd]
    wk_g = wkv[g * P:(g + 1) * P, :]           # [128, 2048]
    wv_g = wkv[KVH * HD + g * P: KVH * HD + (g + 1) * P, :]
    wkv_g = np.concatenate([wk_g, wv_g], 0)    # [256, 2048]
    wkvt = np.ascontiguousarray(
        wkv_g.reshape(2 * P, ND, P).transpose(2, 1, 0))  # [i, dc, 256]
    wo_g = wo[:, g * GD:(g + 1) * GD]          # [2048(n), 512]
    wot = np.ascontiguousarray(
        wo_g.reshape(DIM, G, P).transpose(2, 1, 0))  # [r, h, n]
    cos4 = np.ascontiguousarray(
        np.tile(cos.reshape(NT, P, 1, HD), (1, 1, G, 1)).reshape(NT, P, GD))
    sin4 = np.ascontiguousarray(
        np.tile(sin.reshape(NT, P, 1, HD), (1, 1, G, 1)).reshape(NT, P, GD))
    g128 = np.ascontiguousarray(np.tile(gvec[None, :], (P, 1)))
    return {
        "xt": xt.astype(np.float32), "ctxt": ctxt.astype(np.float32),
        "wqt": wqt.astype(np.float32), "wkvt": wkvt.astype(np.float32),
        "wot": wot.astype(np.float32), "cos4": cos4.astype(np.float32),
        "sin4": sin4.astype(np.float32), "maskb": maskb_col.astype(np.float32),
        "g128": g128.astype(np.float32),
    }


def kernel(x, context, freqs_cos, freqs_sin, context_mask, wq, wkv, wo,
           q_norm_w, k_norm_w, _trace=False):
    x = np.asarray(x, np.float32)
    context = np.asarray(context, np.float32)
    cos = np.asarray(freqs_cos, np.float32).reshape(T, HD)
    sin = np.asarray(freqs_sin, np.float32).reshape(T, HD)
    mask = np.asarray(context_mask).astype(bool)
    wq = np.asarray(wq, np.float32)
    wkv = np.asarray(wkv, np.float32)
    wo = np.asarray(wo, np.float32)
    gvec = (np.asarray(q_norm_w, np.float32) * np.asarray(k_norm_w, np.float32))

    mask_any = mask.any(axis=0)  # union over batches
    active = tuple(si for si in range(NS) if mask_any[si * P:(si + 1) * P].any())
    if not active:
        active = (0,)

    key = active
    if key not in _cache:
        _cache[key] = _build(active)
    nc, names = _cache[key]

    in_maps = []
    for core in range(8):
        b, g = core // KVH, core % KVH
        maskb_col = np.where(mask[b], 0.0, MASK_NEG).astype(np.float32) \
            .reshape(NS, P).T  # [P, NS] column si = chunk si
        m = _prep_core(b, g, x, context, cos, sin,
                       np.ascontiguousarray(maskb_col), wq, wkv, wo, gvec, active)
        in_maps.append({names[k]: v for k, v in m.items()})

    res = run_bass_kernel_spmd(nc, in_maps, core_ids=list(range(8)), trace=_trace)
    out = np.zeros((B, T, DIM), np.float32)
    for core in range(8):
        out[core // KVH] += res.results[core][names["out"]]
    if _trace:
        kernel._last_result = res
    return out
